# revision 8
# baseline (speedup 1.0000x reference)
"""2-layer relational GCN (RGCN) on Trainium2, 8-core SPMD.

Sharding: edges are partitioned by dst-node range (core c owns dst nodes
[c*N/8, (c+1)*N/8)); node features and per-relation weights are replicated.
Self-loops are folded in as an extra relation.

Per core:
  Phase 1 (layer-1 messages): edges grouped by etype; indirect-DMA gather of
    feat[src] rows -> PE transpose -> matmul with W1[r] -> messages scattered
    (indirect DMA) into a dst-sorted message buffer.
  Phase 2 (layer-1 aggregation): per 128-dst-node tile, accumulate messages
    with one-hot selection matmuls in PSUM; +bias, ReLU -> h shard.
  AllGather h shards -> full h on every core.
  Phase 3 (layer 2, fused): gather h[src] rows in dst order, transform by all
    relations at once (W2 flattened), mask-select by etype, one-hot aggregate,
    +bias -> output shard. Host concatenates the 8 shards.
"""

import numpy as np

P = 128          # partitions / tile edge
C = 8            # NeuronCores
GB = 1           # 128-row tiles per indirect-DMA batch (>1 corrupts on HW:
                 # concurrent >128-descriptor indirect DMAs race; see notes)
DEBUG = False    # add msgbuf/h dump outputs
NO_COLLECTIVE = False  # replace AllGather with a local copy (TimelineSim)

_CACHE = {}


# ---------------------------------------------------------------- host prep

def _preprocess(feat, W1, loop1, b1, W2, loop2, b2, src, dst, etype):
    feat = np.ascontiguousarray(np.asarray(feat, dtype=np.float32))
    W1 = np.asarray(W1, dtype=np.float32)
    W2 = np.asarray(W2, dtype=np.float32)
    loop1 = np.asarray(loop1, dtype=np.float32)
    loop2 = np.asarray(loop2, dtype=np.float32)
    b1 = np.asarray(b1, dtype=np.float32)
    b2 = np.asarray(b2, dtype=np.float32)
    src = np.asarray(src).astype(np.int64).ravel()
    dst = np.asarray(dst).astype(np.int64).ravel()
    etype = np.asarray(etype).astype(np.int64).ravel()

    N, D = feat.shape
    R, _, H = W1.shape
    O = W2.shape[2]
    assert D == P and N % C == 0, (N, D)
    S = N // C                       # dst nodes per core
    NT = -(-S // P)                  # node tiles per core
    Rp = R + 1                       # +1 self-loop relation

    # append self-loop edges (relation R)
    sl = np.arange(N, dtype=np.int64)
    asrc = np.concatenate([src, sl])
    adst = np.concatenate([dst, sl])
    aet = np.concatenate([etype, np.full(N, R, dtype=np.int64)])

    core_of = adst // S

    per_core = []
    cnt1 = np.zeros((C, Rp), np.int64)
    cnt2 = np.zeros((C, NT), np.int64)
    for c in range(C):
        m = core_of == c
        es, ed, ee = asrc[m], adst[m], aet[m]
        tl = ed - c * S
        tid = tl // P
        cnt1[c] = np.bincount(ee, minlength=Rp)
        cnt2[c] = np.bincount(tid, minlength=NT)
        per_core.append((es, ee, tl, tid))

    g1cap = (-(-cnt1.max(0) // P)) * P            # padded per-relation sizes
    k_t = np.maximum(1, -(-cnt2.max(0) // P))     # chunks per node tile
    off1 = np.concatenate([[0], np.cumsum(g1cap)])
    off2 = np.concatenate([[0], np.cumsum(k_t * P)])
    L2 = int(off2[-1])                            # phase-2 slot count
    T1 = int(off1[-1]) // P
    # ensure enough phase-1 pad slots to cover every phase-2 pad slot
    extra_tiles = max(0, -(-(L2 - T1 * P) // P))
    T1 += extra_tiles
    T2 = L2 // P

    tile_rel = []
    for r in range(Rp):
        tile_rel += [r] * (int(g1cap[r]) // P)
    tile_rel += [0] * extra_tiles
    chunk_tile = []
    chunk_k = []
    for t in range(NT):
        for k in range(int(k_t[t])):
            chunk_tile.append(t)
            chunk_k.append(k)

    # replicated tensors
    W1e = np.concatenate([W1, loop1[None]], axis=0)          # [Rp, D, H]
    W2e = np.concatenate([W2, loop2[None]], axis=0)          # [Rp, H, O]
    w1f = np.ascontiguousarray(W1e.transpose(1, 0, 2).reshape(D, Rp * H))
    w2f = np.ascontiguousarray(W2e.transpose(1, 0, 2).reshape(H, Rp * O))
    b1b = np.ascontiguousarray(np.broadcast_to(b1, (P, H)))
    b2b = np.ascontiguousarray(np.broadcast_to(b2, (P, O)))

    in_maps = []
    for c in range(C):
        es, ee, tl, tid = per_core[c]
        nE = len(es)
        # phase-1 slots: grouped by etype, sorted by src within a group
        o1 = np.lexsort((es, ee))
        starts1 = np.concatenate([[0], np.cumsum(cnt1[c])[:-1]])
        s_ee = ee[o1]
        slot1_sorted = off1[s_ee] + (np.arange(nE) - starts1[s_ee])
        slot1 = np.empty(nE, np.int64)
        slot1[o1] = slot1_sorted
        # phase-2 slots: grouped by dst tile, sorted by src within a tile
        o2 = np.lexsort((es, tid))
        starts2 = np.concatenate([[0], np.cumsum(cnt2[c])[:-1]])
        s_tid = tid[o2]
        slot2_sorted = off2[s_tid] + (np.arange(nE) - starts2[s_tid])
        slot2 = np.empty(nE, np.int64)
        slot2[o2] = slot2_sorted

        g1 = np.zeros(T1 * P, np.int32)
        s1 = np.zeros(T1 * P, np.int32)
        d2 = np.full(L2, -1.0, np.float32)
        g3 = np.zeros(L2, np.int32)
        e3 = np.full(L2, -1.0, np.float32)
        g1[slot1] = es
        s1[slot1] = slot2
        d2[slot2] = (tl % P).astype(np.float32)
        g3[slot2] = es
        e3[slot2] = ee.astype(np.float32)

        pad1 = np.setdiff1d(np.arange(T1 * P, dtype=np.int64), slot1)
        pad2 = np.setdiff1d(np.arange(L2, dtype=np.int64), slot2)
        assert len(pad1) >= len(pad2), (len(pad1), len(pad2))
        # first cover every phase-2 pad slot (keeps msgbuf fully initialized),
        # dump the remaining phase-1 pads into the spare rows past L2
        n2 = len(pad2)
        if n2:
            s1[pad1[:n2]] = pad2
        rest = pad1[n2:]
        if len(rest):
            s1[rest] = L2 + (np.arange(len(rest)) % P)

        def tr(a, T):
            return np.ascontiguousarray(a.reshape(T, P).T)

        in_maps.append({
            "feat": feat, "w1f": w1f, "w2f": w2f, "b1b": b1b, "b2b": b2b,
            "g1t": tr(g1, T1), "s1t": tr(s1, T1),
            "d2t": tr(d2, T2), "g3t": tr(g3, T2), "e3t": tr(e3, T2),
        })

    plan = dict(N=N, D=D, H=H, O=O, Rp=Rp, S=S, NT=NT, T1=T1, T2=T2,
                tile_rel=tuple(tile_rel), chunk_tile=tuple(chunk_tile),
                chunk_k=tuple(chunk_k), k_t=tuple(int(x) for x in k_t))
    return plan, in_maps


# ---------------------------------------------------------------- device prog

def _bc_inner(ap, n):
    """[P, c] -> [P, c, n], broadcasting the new innermost dim."""
    import concourse.bass as bass
    return bass.AP(ap.tensor, ap.offset, list(ap.ap) + [[0, n]])


def _bc_mid(ap, g):
    """[P, f] -> [P, g, f], broadcasting the new middle dim."""
    import concourse.bass as bass
    a = list(ap.ap)
    return bass.AP(ap.tensor, ap.offset, [a[0], [0, g]] + a[1:])


def _build(plan):
    import concourse.bacc as bacc
    import concourse.tile as tile
    import concourse.mybir as mybir
    from concourse.bass import IndirectOffsetOnAxis
    from concourse.masks import make_identity

    N, D, H, O, Rp = plan["N"], plan["D"], plan["H"], plan["O"], plan["Rp"]
    S, NT, T1, T2 = plan["S"], plan["NT"], plan["T1"], plan["T2"]
    tile_rel, chunk_tile, chunk_k, k_t = (plan["tile_rel"], plan["chunk_tile"],
                                          plan["chunk_k"], plan["k_t"])
    f32 = mybir.dt.float32
    i32 = mybir.dt.int32
    AO = mybir.AluOpType

    nc = bacc.Bacc("TRN2", target_bir_lowering=False, debug=False,
                   num_devices=C)
    feat = nc.dram_tensor("feat", [N, D], f32, kind="ExternalInput")
    w1f = nc.dram_tensor("w1f", [D, Rp * H], f32, kind="ExternalInput")
    w2f = nc.dram_tensor("w2f", [H, Rp * O], f32, kind="ExternalInput")
    b1b = nc.dram_tensor("b1b", [P, H], f32, kind="ExternalInput")
    b2b = nc.dram_tensor("b2b", [P, O], f32, kind="ExternalInput")
    g1t = nc.dram_tensor("g1t", [P, T1], i32, kind="ExternalInput")
    s1t = nc.dram_tensor("s1t", [P, T1], i32, kind="ExternalInput")
    d2t = nc.dram_tensor("d2t", [P, T2], f32, kind="ExternalInput")
    g3t = nc.dram_tensor("g3t", [P, T2], i32, kind="ExternalInput")
    e3t = nc.dram_tensor("e3t", [P, T2], f32, kind="ExternalInput")
    outs = nc.dram_tensor("out_shard", [S, O], f32, kind="ExternalOutput")
    dbg_msg = dbg_h = dbg_hf = None
    if DEBUG:
        dbg_msg = nc.dram_tensor("dbg_msg", [T2 * P + P, H], f32,
                                 kind="ExternalOutput")
        dbg_h = nc.dram_tensor("dbg_h", [S, H], f32, kind="ExternalOutput")
        dbg_hf = nc.dram_tensor("dbg_hf", [N, H], f32, kind="ExternalOutput")

    with tile.TileContext(nc) as tc:
        with tc.tile_pool(name="dram", bufs=1, space="DRAM") as dramp:
            msgbuf = dramp.tile([T2 * P + P, H], f32, name="msgbuf")
            h_shard = dramp.tile([S, H], f32, name="h_shard")
            h_full = dramp.tile([N, H], f32, addr_space="Shared", name="h_full")

            with tc.tile_pool(name="const", bufs=1) as cp:
                ident = cp.tile([P, P], f32, name="ident")
                make_identity(nc, ident[:])
                iota_i = cp.tile([P, P], i32, name="iota_i")
                nc.gpsimd.iota(iota_i[:], pattern=[[1, P]], base=0,
                               channel_multiplier=0)
                iota_f = cp.tile([P, P], f32, name="iota_f")
                nc.vector.tensor_copy(iota_f[:], iota_i[:])
                c40_i = cp.tile([P, Rp * O], i32, name="c40_i")
                nc.gpsimd.iota(c40_i[:], pattern=[[1, Rp], [0, O]], base=0,
                               channel_multiplier=0)
                c40_f = cp.tile([P, Rp * O], f32, name="c40_f")
                nc.vector.tensor_copy(c40_f[:], c40_i[:])
                w1s = cp.tile([D, Rp * H], f32, name="w1s")
                nc.sync.dma_start(out=w1s[:], in_=w1f[:])
                w2s = cp.tile([H, Rp * O], f32, name="w2s")
                nc.sync.dma_start(out=w2s[:], in_=w2f[:])
                b1s = cp.tile([P, H], f32, name="b1s")
                nc.sync.dma_start(out=b1s[:], in_=b1b[:])
                b2s = cp.tile([P, O], f32, name="b2s")
                nc.sync.dma_start(out=b2s[:], in_=b2b[:])
                g1s = cp.tile([P, T1], i32, name="g1s")
                nc.sync.dma_start(out=g1s[:], in_=g1t[:])
                s1s = cp.tile([P, T1], i32, name="s1s")
                nc.sync.dma_start(out=s1s[:], in_=s1t[:])
                d2s = cp.tile([P, T2], f32, name="d2s")
                nc.sync.dma_start(out=d2s[:], in_=d2t[:])
                g3s = cp.tile([P, T2], i32, name="g3s")
                nc.sync.dma_start(out=g3s[:], in_=g3t[:])
                e3s = cp.tile([P, T2], f32, name="e3s")
                nc.sync.dma_start(out=e3s[:], in_=e3t[:])

                # ---------------- phase 1: layer-1 messages -----------------
                with tc.tile_pool(name="p1sb", bufs=6) as sb, \
                     tc.tile_pool(name="p1ps", bufs=2, space="PSUM") as psp:
                    for u0 in range(0, T1, GB):
                        nb = min(GB, T1 - u0)
                        gat = sb.tile([P, nb * D], f32, tag="gat", name="gat")
                        nc.gpsimd.indirect_dma_start(
                            out=gat[:], out_offset=None, in_=feat[:],
                            in_offset=IndirectOffsetOnAxis(
                                ap=g1s[:, u0:u0 + nb], axis=0))
                        gtp = psp.tile([P, nb * P], f32, tag="gtp", name="gtp")
                        msp = psp.tile([P, nb * H], f32, tag="msp", name="msp")
                        stage = sb.tile([P, nb * H], f32, tag="stage",
                                        name="stage")
                        for j in range(nb):
                            r = tile_rel[u0 + j]
                            nc.tensor.transpose(
                                out=gtp[:, j * P:(j + 1) * P],
                                in_=gat[:, j * D:(j + 1) * D],
                                identity=ident[:])
                            gts = sb.tile([P, P], f32, tag="gts", name="gts")
                            if j % 2 == 0:
                                nc.vector.tensor_copy(
                                    gts[:], gtp[:, j * P:(j + 1) * P])
                            else:
                                nc.scalar.copy(
                                    out=gts[:], in_=gtp[:, j * P:(j + 1) * P])
                            nc.tensor.matmul(
                                out=msp[:, j * H:(j + 1) * H], lhsT=gts[:],
                                rhs=w1s[:, r * H:(r + 1) * H],
                                start=True, stop=True)
                        nc.scalar.copy(out=stage[:], in_=msp[:])
                        nc.gpsimd.indirect_dma_start(
                            out=msgbuf[:],
                            out_offset=IndirectOffsetOnAxis(
                                ap=s1s[:, u0:u0 + nb], axis=0),
                            in_=stage[:], in_offset=None)

                # ---------------- phase 2: layer-1 aggregation --------------
                with tc.tile_pool(name="p2sb", bufs=6) as sb2, \
                     tc.tile_pool(name="p2ps", bufs=2, space="PSUM") as ps2:
                    cur_agp = None
                    for u0 in range(0, T2, GB):
                        nb = min(GB, T2 - u0)
                        mch = sb2.tile([P, nb * H], f32, tag="mch", name="mch")
                        nc.sync.dma_start(
                            out=mch[:].rearrange("p (g h) -> p g h", g=nb),
                            in_=msgbuf[u0 * P:(u0 + nb) * P, :].rearrange(
                                "(g p) h -> p g h", p=P))
                        selb = sb2.tile([P, nb * P], f32, tag="selb",
                                        name="selb")
                        nc.vector.tensor_tensor(
                            out=selb[:].rearrange("p (g j) -> p g j", g=nb),
                            in0=_bc_inner(d2s[:, u0:u0 + nb], P),
                            in1=_bc_mid(iota_f[:], nb),
                            op=AO.is_equal)
                        for j in range(nb):
                            t = chunk_tile[u0 + j]
                            k = chunk_k[u0 + j]
                            if k == 0:
                                cur_agp = ps2.tile([P, H], f32, tag="agp",
                                                   name="agp")
                            nc.tensor.matmul(
                                out=cur_agp[:],
                                lhsT=selb[:, j * P:(j + 1) * P],
                                rhs=mch[:, j * H:(j + 1) * H],
                                start=(k == 0), stop=(k == k_t[t] - 1))
                            if k == k_t[t] - 1:
                                hb = sb2.tile([P, H], f32, tag="hb", name="hb")
                                nc.vector.tensor_tensor(
                                    out=hb[:], in0=cur_agp[:], in1=b1s[:],
                                    op=AO.add)
                                nc.vector.tensor_scalar_max(
                                    out=hb[:], in0=hb[:], scalar1=0.0)
                                rows = min(P, S - t * P)
                                nc.sync.dma_start(
                                    out=h_shard[t * P:t * P + rows, :],
                                    in_=hb[:rows, :])
                    if NO_COLLECTIVE:
                        nc.sync.dma_start(out=h_full[0:S, :], in_=h_shard[:])
                    else:
                        nc.gpsimd.collective_compute(
                            "AllGather", AO.bypass,
                            replica_groups=[list(range(C))],
                            ins=[h_shard[:].opt()], outs=[h_full[:].opt()])
                    if DEBUG:
                        nc.sync.dma_start(out=dbg_msg[:], in_=msgbuf[:])
                        nc.sync.dma_start(out=dbg_h[:], in_=h_shard[:])
                        nc.sync.dma_start(out=dbg_hf[:], in_=h_full[:])

                # ---------------- phase 3: layer 2 (fused) ------------------
                with tc.tile_pool(name="p3sb", bufs=6) as sb3, \
                     tc.tile_pool(name="p3ps", bufs=2, space="PSUM") as ps3:
                    cur_otp = None
                    for u0 in range(0, T2, GB):
                        nb = min(GB, T2 - u0)
                        hg = sb3.tile([P, nb * H], f32, tag="hg", name="hg")
                        nc.gpsimd.indirect_dma_start(
                            out=hg[:], out_offset=None, in_=h_full[:],
                            in_offset=IndirectOffsetOnAxis(
                                ap=g3s[:, u0:u0 + nb], axis=0))
                        hgtp = ps3.tile([H, nb * P], f32, tag="hgtp",
                                        name="hgtp")
                        for j in range(nb):
                            nc.tensor.transpose(
                                out=hgtp[:, j * P:(j + 1) * P],
                                in_=hg[:, j * H:(j + 1) * H],
                                identity=ident[:])
                        hgt = sb3.tile([H, nb * P], f32, tag="hgt", name="hgt")
                        nc.scalar.copy(out=hgt[:], in_=hgtp[:])
                        m40 = ps3.tile([P, nb * Rp * O], f32, tag="m40",
                                       name="m40")
                        for j in range(nb):
                            nc.tensor.matmul(
                                out=m40[:, j * Rp * O:(j + 1) * Rp * O],
                                lhsT=hgt[:, j * P:(j + 1) * P], rhs=w2s[:],
                                start=True, stop=True)
                        mskb = sb3.tile([P, nb * Rp * O], f32, tag="mskb",
                                        name="mskb")
                        nc.vector.tensor_tensor(
                            out=mskb[:].rearrange("p (g c) -> p g c", g=nb),
                            in0=_bc_inner(e3s[:, u0:u0 + nb], Rp * O),
                            in1=_bc_mid(c40_f[:], nb),
                            op=AO.is_equal)
                        nc.vector.tensor_tensor(
                            out=mskb[:], in0=mskb[:], in1=m40[:], op=AO.mult)
                        m2b = sb3.tile([P, nb * O], f32, tag="m2b", name="m2b")
                        nc.vector.tensor_reduce(
                            out=m2b[:],
                            in_=mskb[:].rearrange("p (g r o) -> p g o r",
                                                  g=nb, r=Rp, o=O),
                            axis=mybir.AxisListType.X, op=AO.add)
                        sel2b = sb3.tile([P, nb * P], f32, tag="sel2b",
                                         name="sel2b")
                        nc.vector.tensor_tensor(
                            out=sel2b[:].rearrange("p (g j) -> p g j", g=nb),
                            in0=_bc_inner(d2s[:, u0:u0 + nb], P),
                            in1=_bc_mid(iota_f[:], nb),
                            op=AO.is_equal)
                        for j in range(nb):
                            t = chunk_tile[u0 + j]
                            k = chunk_k[u0 + j]
                            if k == 0:
                                cur_otp = ps3.tile([P, O], f32, tag="otp",
                                                   name="otp")
                            nc.tensor.matmul(
                                out=cur_otp[:],
                                lhsT=sel2b[:, j * P:(j + 1) * P],
                                rhs=m2b[:, j * O:(j + 1) * O],
                                start=(k == 0), stop=(k == k_t[t] - 1))
                            if k == k_t[t] - 1:
                                ob = sb3.tile([P, O], f32, tag="ob", name="ob")
                                nc.vector.tensor_tensor(
                                    out=ob[:], in0=cur_otp[:], in1=b2s[:],
                                    op=AO.add)
                                rows = min(P, S - t * P)
                                nc.sync.dma_start(
                                    out=outs[t * P:t * P + rows, :],
                                    in_=ob[:rows, :])

    nc.compile()
    return nc


# ---------------------------------------------------------------- entry

def _run(in_maps, plan, trace=False):
    from concourse.bass_utils import run_bass_kernel_spmd

    key = (plan["N"], plan["T1"], plan["T2"], plan["tile_rel"], plan["k_t"],
           GB, DEBUG)
    nc = _CACHE.get(key)
    if nc is None:
        nc = _build(plan)
        _CACHE[key] = nc
    res = run_bass_kernel_spmd(nc, in_maps, list(range(C)), trace=trace)
    out = np.concatenate([res.results[c]["out_shard"] for c in range(C)],
                         axis=0)
    return out, res


def kernel(**inputs):
    plan, in_maps = _preprocess(**inputs)
    out, _ = _run(in_maps, plan)
    return out


def kernel_traced(**inputs):
    plan, in_maps = _preprocess(**inputs)
    return _run(in_maps, plan, trace=True)


# revision 9
# speedup vs baseline: 1.2607x; 1.2607x over previous
"""2-layer relational GCN (RGCN) on Trainium2, 8-core SPMD.

Sharding: edges are partitioned by dst-node range (core c owns dst nodes
[c*N/8, (c+1)*N/8)); node features and per-relation weights are replicated.
Self-loops are folded in as an extra relation.

Per core:
  Phase 1 (layer-1 messages): edges grouped by etype; indirect-DMA gather of
    feat[src] rows -> PE transpose -> matmul with W1[r] -> messages scattered
    (indirect DMA) into a dst-sorted message buffer.
  Phase 2 (layer-1 aggregation): per 128-dst-node tile, accumulate messages
    with one-hot selection matmuls in PSUM; +bias, ReLU -> h shard.
  AllGather h shards -> full h on every core.
  Phase 3 (layer 2, fused): gather h[src] rows in dst order, transform by all
    relations at once (W2 flattened), mask-select by etype, one-hot aggregate,
    +bias -> output shard. Host concatenates the 8 shards.
"""

import numpy as np

P = 128          # partitions / tile edge
C = 8            # NeuronCores
GB = 1           # 128-row tiles per indirect-DMA batch (>1 corrupts on HW:
                 # concurrent >128-descriptor indirect DMAs race; see notes)
DEBUG = False    # add msgbuf/h dump outputs
NO_COLLECTIVE = False  # replace AllGather with a local copy (TimelineSim)

_CACHE = {}


# ---------------------------------------------------------------- host prep

def _preprocess(feat, W1, loop1, b1, W2, loop2, b2, src, dst, etype):
    feat = np.ascontiguousarray(np.asarray(feat, dtype=np.float32))
    W1 = np.asarray(W1, dtype=np.float32)
    W2 = np.asarray(W2, dtype=np.float32)
    loop1 = np.asarray(loop1, dtype=np.float32)
    loop2 = np.asarray(loop2, dtype=np.float32)
    b1 = np.asarray(b1, dtype=np.float32)
    b2 = np.asarray(b2, dtype=np.float32)
    src = np.asarray(src).astype(np.int64).ravel()
    dst = np.asarray(dst).astype(np.int64).ravel()
    etype = np.asarray(etype).astype(np.int64).ravel()

    N, D = feat.shape
    R, _, H = W1.shape
    O = W2.shape[2]
    assert D == P and N % C == 0, (N, D)
    S = N // C                       # dst nodes per core
    NT = -(-S // P)                  # node tiles per core
    Rp = R + 1                       # +1 self-loop relation

    # append self-loop edges (relation R)
    sl = np.arange(N, dtype=np.int64)
    asrc = np.concatenate([src, sl])
    adst = np.concatenate([dst, sl])
    aet = np.concatenate([etype, np.full(N, R, dtype=np.int64)])

    core_of = adst // S

    per_core = []
    cnt1 = np.zeros((C, Rp), np.int64)
    cnt2 = np.zeros((C, NT), np.int64)
    for c in range(C):
        m = core_of == c
        es, ed, ee = asrc[m], adst[m], aet[m]
        tl = ed - c * S
        tid = tl // P
        cnt1[c] = np.bincount(ee, minlength=Rp)
        cnt2[c] = np.bincount(tid, minlength=NT)
        per_core.append((es, ee, tl, tid))

    g1cap = (-(-cnt1.max(0) // P)) * P            # padded per-relation sizes
    k_t = np.maximum(1, -(-cnt2.max(0) // P))     # chunks per node tile
    off1 = np.concatenate([[0], np.cumsum(g1cap)])
    off2 = np.concatenate([[0], np.cumsum(k_t * P)])
    L2 = int(off2[-1])                            # phase-2 slot count
    T1 = int(off1[-1]) // P
    # ensure enough phase-1 pad slots to cover every phase-2 pad slot
    extra_tiles = max(0, -(-(L2 - T1 * P) // P))
    T1 += extra_tiles
    T2 = L2 // P

    tile_rel = []
    for r in range(Rp):
        tile_rel += [r] * (int(g1cap[r]) // P)
    tile_rel += [0] * extra_tiles
    chunk_tile = []
    chunk_k = []
    for t in range(NT):
        for k in range(int(k_t[t])):
            chunk_tile.append(t)
            chunk_k.append(k)

    # replicated tensors
    W1e = np.concatenate([W1, loop1[None]], axis=0)          # [Rp, D, H]
    W2e = np.concatenate([W2, loop2[None]], axis=0)          # [Rp, H, O]
    w1f = np.ascontiguousarray(W1e.transpose(1, 0, 2).reshape(D, Rp * H))
    w2f = np.ascontiguousarray(W2e.transpose(1, 0, 2).reshape(H, Rp * O))
    b1b = np.ascontiguousarray(np.broadcast_to(b1, (P, H)))
    b2b = np.ascontiguousarray(np.broadcast_to(b2, (P, O)))

    in_maps = []
    for c in range(C):
        es, ee, tl, tid = per_core[c]
        nE = len(es)
        # phase-1 slots: grouped by etype, sorted by src within a group
        o1 = np.lexsort((es, ee))
        starts1 = np.concatenate([[0], np.cumsum(cnt1[c])[:-1]])
        s_ee = ee[o1]
        slot1_sorted = off1[s_ee] + (np.arange(nE) - starts1[s_ee])
        slot1 = np.empty(nE, np.int64)
        slot1[o1] = slot1_sorted
        # phase-2 slots: grouped by dst tile, sorted by src within a tile
        o2 = np.lexsort((es, tid))
        starts2 = np.concatenate([[0], np.cumsum(cnt2[c])[:-1]])
        s_tid = tid[o2]
        slot2_sorted = off2[s_tid] + (np.arange(nE) - starts2[s_tid])
        slot2 = np.empty(nE, np.int64)
        slot2[o2] = slot2_sorted

        g1 = np.zeros(T1 * P, np.int32)
        s1 = np.zeros(T1 * P, np.int32)
        d2 = np.full(L2, -1.0, np.float32)
        g3 = np.zeros(L2, np.int32)
        e3 = np.full(L2, -1.0, np.float32)
        g1[slot1] = es
        s1[slot1] = slot2
        d2[slot2] = (tl % P).astype(np.float32)
        g3[slot2] = es
        e3[slot2] = ee.astype(np.float32)

        pad1 = np.setdiff1d(np.arange(T1 * P, dtype=np.int64), slot1)
        pad2 = np.setdiff1d(np.arange(L2, dtype=np.int64), slot2)
        assert len(pad1) >= len(pad2), (len(pad1), len(pad2))
        # first cover every phase-2 pad slot (keeps msgbuf fully initialized),
        # dump the remaining phase-1 pads into the spare rows past L2
        n2 = len(pad2)
        if n2:
            s1[pad1[:n2]] = pad2
        rest = pad1[n2:]
        if len(rest):
            s1[rest] = L2 + (np.arange(len(rest)) % P)

        def tr(a, T):
            return np.ascontiguousarray(a.reshape(T, P).T)

        in_maps.append({
            "feat": feat, "w1f": w1f, "w2f": w2f, "b1b": b1b, "b2b": b2b,
            "g1t": tr(g1, T1), "s1t": tr(s1, T1),
            "d2t": tr(d2, T2), "g3t": tr(g3, T2), "e3t": tr(e3, T2),
        })

    plan = dict(N=N, D=D, H=H, O=O, Rp=Rp, S=S, NT=NT, T1=T1, T2=T2,
                tile_rel=tuple(tile_rel), chunk_tile=tuple(chunk_tile),
                chunk_k=tuple(chunk_k), k_t=tuple(int(x) for x in k_t))
    return plan, in_maps


# ---------------------------------------------------------------- device prog

def _bc_inner(ap, n):
    """[P, c] -> [P, c, n], broadcasting the new innermost dim."""
    import concourse.bass as bass
    return bass.AP(ap.tensor, ap.offset, list(ap.ap) + [[0, n]])


def _bc_mid(ap, g):
    """[P, f] -> [P, g, f], broadcasting the new middle dim."""
    import concourse.bass as bass
    a = list(ap.ap)
    return bass.AP(ap.tensor, ap.offset, [a[0], [0, g]] + a[1:])


def _build(plan):
    import concourse.bacc as bacc
    import concourse.tile as tile
    import concourse.mybir as mybir
    from concourse.bass import IndirectOffsetOnAxis
    from concourse.masks import make_identity

    N, D, H, O, Rp = plan["N"], plan["D"], plan["H"], plan["O"], plan["Rp"]
    S, NT, T1, T2 = plan["S"], plan["NT"], plan["T1"], plan["T2"]
    tile_rel, chunk_tile, chunk_k, k_t = (plan["tile_rel"], plan["chunk_tile"],
                                          plan["chunk_k"], plan["k_t"])
    f32 = mybir.dt.float32
    i32 = mybir.dt.int32
    AO = mybir.AluOpType

    nc = bacc.Bacc("TRN2", target_bir_lowering=False, debug=False,
                   num_devices=C)
    feat = nc.dram_tensor("feat", [N, D], f32, kind="ExternalInput")
    w1f = nc.dram_tensor("w1f", [D, Rp * H], f32, kind="ExternalInput")
    w2f = nc.dram_tensor("w2f", [H, Rp * O], f32, kind="ExternalInput")
    b1b = nc.dram_tensor("b1b", [P, H], f32, kind="ExternalInput")
    b2b = nc.dram_tensor("b2b", [P, O], f32, kind="ExternalInput")
    g1t = nc.dram_tensor("g1t", [P, T1], i32, kind="ExternalInput")
    s1t = nc.dram_tensor("s1t", [P, T1], i32, kind="ExternalInput")
    d2t = nc.dram_tensor("d2t", [P, T2], f32, kind="ExternalInput")
    g3t = nc.dram_tensor("g3t", [P, T2], i32, kind="ExternalInput")
    e3t = nc.dram_tensor("e3t", [P, T2], f32, kind="ExternalInput")
    outs = nc.dram_tensor("out_shard", [S, O], f32, kind="ExternalOutput")
    dbg_msg = dbg_h = dbg_hf = None
    if DEBUG:
        dbg_msg = nc.dram_tensor("dbg_msg", [T2 * P + P, H], f32,
                                 kind="ExternalOutput")
        dbg_h = nc.dram_tensor("dbg_h", [S, H], f32, kind="ExternalOutput")
        dbg_hf = nc.dram_tensor("dbg_hf", [N, H], f32, kind="ExternalOutput")

    with tile.TileContext(nc) as tc:
        with tc.tile_pool(name="dram", bufs=1, space="DRAM") as dramp:
            msgbuf = dramp.tile([T2 * P + P, H], f32, name="msgbuf")
            h_shard = dramp.tile([S, H], f32, name="h_shard")
            h_full = dramp.tile([N, H], f32, addr_space="Shared", name="h_full")

            with tc.tile_pool(name="const", bufs=1) as cp:
                ident = cp.tile([P, P], f32, name="ident")
                make_identity(nc, ident[:])
                iota_i = cp.tile([P, P], i32, name="iota_i")
                nc.gpsimd.iota(iota_i[:], pattern=[[1, P]], base=0,
                               channel_multiplier=0)
                iota_f = cp.tile([P, P], f32, name="iota_f")
                nc.vector.tensor_copy(iota_f[:], iota_i[:])
                c40_i = cp.tile([P, Rp * O], i32, name="c40_i")
                nc.gpsimd.iota(c40_i[:], pattern=[[1, Rp], [0, O]], base=0,
                               channel_multiplier=0)
                c40_f = cp.tile([P, Rp * O], f32, name="c40_f")
                nc.vector.tensor_copy(c40_f[:], c40_i[:])
                w1s = cp.tile([D, Rp * H], f32, name="w1s")
                nc.sync.dma_start(out=w1s[:], in_=w1f[:])
                w2s = cp.tile([H, Rp * O], f32, name="w2s")
                nc.sync.dma_start(out=w2s[:], in_=w2f[:])
                b1s = cp.tile([P, H], f32, name="b1s")
                nc.sync.dma_start(out=b1s[:], in_=b1b[:])
                b2s = cp.tile([P, O], f32, name="b2s")
                nc.sync.dma_start(out=b2s[:], in_=b2b[:])
                g1s = cp.tile([P, T1], i32, name="g1s")
                nc.sync.dma_start(out=g1s[:], in_=g1t[:])
                s1s = cp.tile([P, T1], i32, name="s1s")
                nc.sync.dma_start(out=s1s[:], in_=s1t[:])
                d2s = cp.tile([P, T2], f32, name="d2s")
                nc.sync.dma_start(out=d2s[:], in_=d2t[:])
                g3s = cp.tile([P, T2], i32, name="g3s")
                nc.sync.dma_start(out=g3s[:], in_=g3t[:])
                e3s = cp.tile([P, T2], f32, name="e3s")
                nc.sync.dma_start(out=e3s[:], in_=e3t[:])

                # ---------------- phase 1: layer-1 messages -----------------
                with tc.tile_pool(name="p1sb", bufs=6) as sb, \
                     tc.tile_pool(name="p1ps", bufs=2, space="PSUM") as psp:
                    for u0 in range(0, T1, GB):
                        nb = min(GB, T1 - u0)
                        gat = sb.tile([P, nb * D], f32, tag="gat", name="gat")
                        nc.gpsimd.indirect_dma_start(
                            out=gat[:], out_offset=None, in_=feat[:],
                            in_offset=IndirectOffsetOnAxis(
                                ap=g1s[:, u0:u0 + nb], axis=0))
                        gtp = psp.tile([P, nb * P], f32, tag="gtp", name="gtp")
                        msp = psp.tile([P, nb * H], f32, tag="msp", name="msp")
                        stage = sb.tile([P, nb * H], f32, tag="stage",
                                        name="stage")
                        for j in range(nb):
                            r = tile_rel[u0 + j]
                            nc.tensor.transpose(
                                out=gtp[:, j * P:(j + 1) * P],
                                in_=gat[:, j * D:(j + 1) * D],
                                identity=ident[:])
                            gts = sb.tile([P, P], f32, tag="gts", name="gts")
                            if j % 2 == 0:
                                nc.vector.tensor_copy(
                                    gts[:], gtp[:, j * P:(j + 1) * P])
                            else:
                                nc.scalar.copy(
                                    out=gts[:], in_=gtp[:, j * P:(j + 1) * P])
                            nc.tensor.matmul(
                                out=msp[:, j * H:(j + 1) * H], lhsT=gts[:],
                                rhs=w1s[:, r * H:(r + 1) * H],
                                start=True, stop=True)
                        nc.scalar.copy(out=stage[:], in_=msp[:])
                        nc.gpsimd.indirect_dma_start(
                            out=msgbuf[:],
                            out_offset=IndirectOffsetOnAxis(
                                ap=s1s[:, u0:u0 + nb], axis=0),
                            in_=stage[:], in_offset=None)

                # ---------------- phase 2: layer-1 aggregation --------------
                with tc.tile_pool(name="p2sb", bufs=6) as sb2, \
                     tc.tile_pool(name="p2ps", bufs=2, space="PSUM") as ps2:
                    cur_agp = None
                    for u0 in range(0, T2, GB):
                        nb = min(GB, T2 - u0)
                        mch = sb2.tile([P, nb * H], f32, tag="mch", name="mch")
                        nc.sync.dma_start(
                            out=mch[:].rearrange("p (g h) -> p g h", g=nb),
                            in_=msgbuf[u0 * P:(u0 + nb) * P, :].rearrange(
                                "(g p) h -> p g h", p=P))
                        selb = sb2.tile([P, nb * P], f32, tag="selb",
                                        name="selb")
                        nc.vector.tensor_tensor(
                            out=selb[:].rearrange("p (g j) -> p g j", g=nb),
                            in0=_bc_inner(d2s[:, u0:u0 + nb], P),
                            in1=_bc_mid(iota_f[:], nb),
                            op=AO.is_equal)
                        for j in range(nb):
                            t = chunk_tile[u0 + j]
                            k = chunk_k[u0 + j]
                            if k == 0:
                                cur_agp = ps2.tile([P, H], f32, tag="agp",
                                                   name="agp")
                            nc.tensor.matmul(
                                out=cur_agp[:],
                                lhsT=selb[:, j * P:(j + 1) * P],
                                rhs=mch[:, j * H:(j + 1) * H],
                                start=(k == 0), stop=(k == k_t[t] - 1))
                            if k == k_t[t] - 1:
                                hb = sb2.tile([P, H], f32, tag="hb", name="hb")
                                nc.vector.tensor_tensor(
                                    out=hb[:], in0=cur_agp[:], in1=b1s[:],
                                    op=AO.add)
                                nc.vector.tensor_scalar_max(
                                    out=hb[:], in0=hb[:], scalar1=0.0)
                                rows = min(P, S - t * P)
                                nc.sync.dma_start(
                                    out=h_shard[t * P:t * P + rows, :],
                                    in_=hb[:rows, :])
                    if NO_COLLECTIVE:
                        nc.sync.dma_start(out=h_full[0:S, :], in_=h_shard[:])
                    else:
                        nc.gpsimd.collective_compute(
                            "AllGather", AO.bypass,
                            replica_groups=[list(range(C))],
                            ins=[h_shard[:].opt()], outs=[h_full[:].opt()])
                    if DEBUG:
                        nc.sync.dma_start(out=dbg_msg[:], in_=msgbuf[:])
                        nc.sync.dma_start(out=dbg_h[:], in_=h_shard[:])
                        nc.sync.dma_start(out=dbg_hf[:], in_=h_full[:])

                # ---------------- phase 3: layer 2 (fused) ------------------
                with tc.tile_pool(name="p3sb", bufs=6) as sb3, \
                     tc.tile_pool(name="p3ps", bufs=2, space="PSUM") as ps3:
                    cur_otp = None
                    for u0 in range(0, T2, GB):
                        nb = min(GB, T2 - u0)
                        hg = sb3.tile([P, nb * H], f32, tag="hg", name="hg")
                        nc.gpsimd.indirect_dma_start(
                            out=hg[:], out_offset=None, in_=h_full[:],
                            in_offset=IndirectOffsetOnAxis(
                                ap=g3s[:, u0:u0 + nb], axis=0))
                        hgtp = ps3.tile([H, nb * P], f32, tag="hgtp",
                                        name="hgtp")
                        for j in range(nb):
                            nc.tensor.transpose(
                                out=hgtp[:, j * P:(j + 1) * P],
                                in_=hg[:, j * H:(j + 1) * H],
                                identity=ident[:])
                        hgt = sb3.tile([H, nb * P], f32, tag="hgt", name="hgt")
                        nc.scalar.copy(out=hgt[:], in_=hgtp[:])
                        m40 = ps3.tile([P, nb * Rp * O], f32, tag="m40",
                                       name="m40")
                        for j in range(nb):
                            nc.tensor.matmul(
                                out=m40[:, j * Rp * O:(j + 1) * Rp * O],
                                lhsT=hgt[:, j * P:(j + 1) * P], rhs=w2s[:],
                                start=True, stop=True)
                        mskb = sb3.tile([P, nb * Rp * O], f32, tag="mskb",
                                        name="mskb")
                        nc.vector.tensor_tensor(
                            out=mskb[:].rearrange("p (g c) -> p g c", g=nb),
                            in0=_bc_inner(e3s[:, u0:u0 + nb], Rp * O),
                            in1=_bc_mid(c40_f[:], nb),
                            op=AO.is_equal)
                        nc.vector.tensor_tensor(
                            out=mskb[:], in0=mskb[:], in1=m40[:], op=AO.mult)
                        m2b = sb3.tile([P, nb * O], f32, tag="m2b", name="m2b")
                        nc.vector.tensor_reduce(
                            out=m2b[:],
                            in_=mskb[:].rearrange("p (g r o) -> p g o r",
                                                  g=nb, r=Rp, o=O),
                            axis=mybir.AxisListType.X, op=AO.add)
                        sel2b = sb3.tile([P, nb * P], f32, tag="sel2b",
                                         name="sel2b")
                        nc.vector.tensor_tensor(
                            out=sel2b[:].rearrange("p (g j) -> p g j", g=nb),
                            in0=_bc_inner(d2s[:, u0:u0 + nb], P),
                            in1=_bc_mid(iota_f[:], nb),
                            op=AO.is_equal)
                        for j in range(nb):
                            t = chunk_tile[u0 + j]
                            k = chunk_k[u0 + j]
                            if k == 0:
                                cur_otp = ps3.tile([P, O], f32, tag="otp",
                                                   name="otp")
                            nc.tensor.matmul(
                                out=cur_otp[:],
                                lhsT=sel2b[:, j * P:(j + 1) * P],
                                rhs=m2b[:, j * O:(j + 1) * O],
                                start=(k == 0), stop=(k == k_t[t] - 1))
                            if k == k_t[t] - 1:
                                ob = sb3.tile([P, O], f32, tag="ob", name="ob")
                                nc.vector.tensor_tensor(
                                    out=ob[:], in0=cur_otp[:], in1=b2s[:],
                                    op=AO.add)
                                rows = min(P, S - t * P)
                                nc.sync.dma_start(
                                    out=outs[t * P:t * P + rows, :],
                                    in_=ob[:rows, :])

    nc.compile()
    return nc


# ---------------------------------------------------------------- entry

def _run(in_maps, plan, trace=False):
    from concourse.bass_utils import run_bass_kernel_spmd

    key = (plan["N"], plan["T1"], plan["T2"], plan["tile_rel"], plan["k_t"],
           GB, DEBUG)
    nc = _CACHE.get(key)
    if nc is None:
        nc = _build(plan)
        _CACHE[key] = nc
    res = run_bass_kernel_spmd(nc, in_maps, list(range(C)), trace=trace)
    out = np.concatenate([res.results[c]["out_shard"] for c in range(C)],
                         axis=0)
    return out, res


def kernel(**inputs):
    plan, in_maps = _preprocess(**inputs)
    out, _ = _run(in_maps, plan)
    return out
